# revision 24
# baseline (speedup 1.0000x reference)
"""Trainium2 Bass kernel: Bahdanau (additive) attention with coverage.

Reference computation (per batch element b, data-parallel over B=8 cores):
    enc   = tanh(enc_raw + cov[:,None]*wcov)            [S,H]
    a1    = dec @ Wq + bq                               [T,H]
    a2    = enc @ Wc                                    [S,H]
    scores[t,s] = sum_h v[h] * tanh(a1[t,h] + a2[s,h])  [T,S]
    align = softmax(scores, -1)                         [T,S]
    c     = align @ enc                                 [T,H]
    attn_h = [c, dec] @ Wo + bo                         [T,H]
Outputs: attn_h -> [T,B,H], align -> [T,B,S].

Device strategy: cell-factorized tanh. Quantize x = a1 onto C=9 centers
(spacing DELTA), tau = tanh(x - ctr), P = tanh(y + ctr); then exactly
    tanh(x+y) = P + tau*W - tau^2*P*W + tau^3*P^2*W - ...   (W = 1-P^2)
truncated at tau^3 (max |tau| ~ tanh(DELTA/2) -> err ~5e-3 on scores,
align rel err ~5e-3 end-to-end, tolerance 2e-2). Each (cell, k) term is a
rank-1-in-h product of a LEFT tile (mask*v*tau^k over [h,t]) and a RIGHT
tile (P-polynomial over [h,s]), contracted over h by PE into transposed
scoresT[s,t] (out free = 64). ACT does 2 passes/cell (Tanh, Square),
DVE 3 products/cell + small left chains, GPSIMD does coverage adds and
PSUM->SBUF copies. All feature tiles fp16 (DVE 2x/4x modes, 1-cyc PE).
"""

import os

import numpy as np

T, B, S, H = 64, 8, 512, 512
P = 128
KT = H // P   # 4 partition chunks of H
NSC = S // P  # 4 partition chunks of S

C_CELLS = int(os.environ.get("ATTN_CELLS", "9"))
DELTA = float(os.environ.get("ATTN_DELTA", "1.0"))
MAGIC = float(1.5 * 2 ** 23)  # fp32 round-to-nearest-int via add/sub

_BUILT = {}
LAST_RESULT = None


def _emit(nc, tc, ctx, din, dout):
    import concourse.mybir as mybir

    f32 = mybir.dt.float32
    f16 = mybir.dt.float16
    AF = mybir.ActivationFunctionType
    ALU = mybir.AluOpType

    pers = ctx.enter_context(tc.tile_pool(name="pers", bufs=1))
    rt = ctx.enter_context(tc.tile_pool(name="rt", bufs=3))    # right tiles
    lt = ctx.enter_context(tc.tile_pool(name="lt", bufs=3))    # left tiles
    psT = ctx.enter_context(tc.tile_pool(name="psT", bufs=2, space="PSUM"))
    psSm = ctx.enter_context(tc.tile_pool(name="psSm", bufs=2, space="PSUM"))
    psOut = ctx.enter_context(tc.tile_pool(name="psOut", bufs=1, space="PSUM"))

    def ld(name, shape, dt):
        t = pers.tile(shape, dt, tag=name)
        nc.sync.dma_start(out=t[:], in_=din[name][:])
        return t

    # DMA order = need order.
    covr16 = ld("cov16", [1, S], f16)
    wcovr16 = ld("wcov16", [1, H], f16)
    encT16 = ld("encT16", [P, KT * S], f16)
    wc16 = ld("wc16", [P, KT * H], f16)
    decT16 = ld("decT16", [P, KT * T], f16)
    wq16 = ld("wq16", [P, KT * H], f16)
    bqr16 = ld("bq16", [1, H], f16)
    vrep16 = ld("vrep16", [P, KT * T], f16)
    enc16 = ld("enc16", [P, KT * H], f16)
    wo16 = ld("wo16", [P, 2 * KT * H], f16)
    bor16 = ld("bo16", [1, H], f16)
    eye64 = ld("eye64", [T, T], f32)
    eye128 = ld("eye128", [P, P], f32)
    ones16 = pers.tile([1, T], f16, tag="ones16")
    nc.vector.memset(ones16[:], 1.0)
    ones128 = pers.tile([1, P], f16, tag="ones128")
    nc.vector.memset(ones128[:], 1.0)
    # PE p-state warmup: ~3us of dependency-free junk matmuls so the real
    # prologue matmuls run at full clock
    warm = psT.tile([T, T], f32, tag="pt")
    for _ in range(18):
        nc.tensor.matmul(warm[:], ones16[0:1, :], ones16[0:1, :],
                         start=True, stop=True)

    # coverage in [H,S] layout: encT_t = tanh(encT + wcov (x) cov)
    # (adds on GPSIMD to keep DVE free)
    encT_t = pers.tile([P, KT * S], f16, tag="encT_t")
    for i in range(KT):
        op = psT.tile([P, S], f32, tag="pt")
        nc.tensor.matmul(op[:], wcovr16[0:1, i * P:(i + 1) * P], covr16[0:1, :],
                         start=True, stop=True)
        nc.vector.tensor_add(encT16[:, i * S:(i + 1) * S],
                             encT16[:, i * S:(i + 1) * S], op[:])
        nc.scalar.activation(encT_t[:, i * S:(i + 1) * S],
                             encT16[:, i * S:(i + 1) * S], AF.Tanh)

    # a2T[hout, (k,s)] merged fp16 tile
    a2T = pers.tile([P, KT * S], f16, tag="a2T")
    for m in range(KT):
        pm2 = psT.tile([P, S], f32, tag="pt")
        for k in range(KT):
            nc.tensor.matmul(pm2[:], wc16[:, k * H + m * P:k * H + (m + 1) * P],
                             encT_t[:, k * S:(k + 1) * S],
                             start=(k == 0), stop=(k == KT - 1))
        nc.vector.tensor_copy(a2T[:, m * S:(m + 1) * S], pm2[:])

    # a1T[hout, (k,t)] f32 (feeds cell quantization)
    a1T = pers.tile([P, KT * T], f32, tag="a1T")
    for m in range(KT):
        pm1 = psSm.tile([P, T], f32, tag="ps")
        for k in range(KT):
            nc.tensor.matmul(pm1[:], wq16[:, k * H + m * P:k * H + (m + 1) * P],
                             decT16[:, k * T:(k + 1) * T],
                             start=(k == 0), stop=False)
        nc.tensor.matmul(pm1[:], bqr16[0:1, m * P:(m + 1) * P], ones16[0:1, :],
                         start=False, stop=True)
        nc.vector.tensor_copy(a1T[:, m * T:(m + 1) * T], pm1[:])

    # coverage in [S,H] layout (for the c-matmul): enc_t = tanh(enc + cov (x) wcov)
    enc_t = pers.tile([P, KT * H], f16, tag="enc_t")
    for j in range(NSC):
        op = psT.tile([P, H], f32, tag="pt")
        nc.tensor.matmul(op[:], covr16[0:1, j * P:(j + 1) * P], wcovr16[0:1, :],
                         start=True, stop=True)
        nc.vector.tensor_add(enc16[:, j * H:(j + 1) * H],
                             enc16[:, j * H:(j + 1) * H], op[:])
        nc.scalar.activation(enc_t[:, j * H:(j + 1) * H],
                             enc16[:, j * H:(j + 1) * H], AF.Tanh)

    # attn_h dec-part (independent of the attention loop): start pa early
    pa = psOut.tile([T, H], f32, tag="pa")
    for k in range(KT):
        nc.tensor.matmul(pa[:], decT16[:, k * T:(k + 1) * T],
                         wo16[:, (KT + k) * H:(KT + k + 1) * H],
                         start=(k == 0), stop=False)
    nc.tensor.matmul(pa[:], ones16[0:1, :], bor16[0:1, :], start=False,
                     stop=False)

    # ---- left-side quantization: cellf = clamp(round(a1/DELTA)), taum = tanh(ctr-a1)
    CH = (C_CELLS - 1) // 2  # centers at DELTA*(-CH..CH)
    if DELTA != 1.0:
        r1 = pers.tile([P, KT * T], f32, tag="r1")
        nc.vector.tensor_scalar(r1[:], a1T[:], float(1.0 / DELTA), None, ALU.mult)
    else:
        r1 = a1T
    cellf = pers.tile([P, KT * T], f32, tag="cellf")
    nc.vector.tensor_scalar(cellf[:], r1[:], MAGIC, MAGIC, ALU.add, ALU.subtract)
    nc.vector.tensor_scalar(cellf[:], cellf[:], float(CH), float(-CH),
                            ALU.min, ALU.max)
    negd = pers.tile([P, KT * T], f32, tag="negd")
    nc.vector.scalar_tensor_tensor(negd[:], cellf[:], float(DELTA), a1T[:],
                                   ALU.mult, ALU.subtract)  # ctr - a1 = -delta
    taum = pers.tile([P, KT * T], f16, tag="taum")
    nc.scalar.activation(taum[:], negd[:], AF.Tanh)
    # global left power tiles u_k = vrep * taum^k (small DVE ops)
    u0 = vrep16
    u1 = pers.tile([P, KT * T], f16, tag="u1")
    nc.vector.tensor_tensor(u1[:], u0[:], taum[:], ALU.mult)
    u2 = pers.tile([P, KT * T], f16, tag="u2")
    nc.vector.tensor_tensor(u2[:], u1[:], taum[:], ALU.mult)
    u3 = pers.tile([P, KT * T], f16, tag="u3")
    nc.vector.tensor_tensor(u3[:], u2[:], taum[:], ALU.mult)
    upow = [u0, u1, u2, u3]

    # ---- scoresT accumulator; zero the bank once
    scoresT = psOut.tile([P, NSC * T], f32, tag="scT")
    zrow = pers.tile([1, NSC * T], f16, tag="zrow")
    nc.vector.memset(zrow[:], 0.0)
    nc.tensor.matmul(scoresT[:], ones128[0:1, :], zrow[0:1, :],
                     start=True, stop=False)

    # ---- per-cell features + PE contraction
    # series: tanh(x+y) = P + tau*W - tau^2 P W + tau^3 P^2 W   (W = 1-P^2)
    # with taum = -tau and Wm = P^2-1 = -W the pairs are:
    #   (mv, P), (mv*taum, Wm), (mv*taum^2, P*Wm), (mv*taum^3, P^2*Wm)
    n_cells = C_CELLS
    ctrb = pers.tile([P, n_cells], f32, tag="ctrb")
    for ci in range(n_cells):
        nc.vector.memset(ctrb[:, ci:ci + 1], float(DELTA * (ci - CH)))
    for ci in range(n_cells):
        Pt = rt.tile([P, KT * S], f16, tag="P", name=f"P{ci}")
        nc.scalar.activation(Pt[:], a2T[:], AF.Tanh, bias=ctrb[:, ci:ci + 1])
        P2 = rt.tile([P, KT * S], f16, tag="P2", name=f"P2{ci}")
        nc.scalar.activation(P2[:], Pt[:], AF.Square)
        Wm = rt.tile([P, KT * S], f16, tag="Wm", name=f"Wm{ci}")
        nc.vector.tensor_scalar(Wm[:], P2[:], 1.0, 1.0, ALU.mult, ALU.subtract)
        PWm = rt.tile([P, KT * S], f16, tag="PWm", name=f"PWm{ci}")
        nc.vector.tensor_tensor(PWm[:], Pt[:], Wm[:], ALU.mult)
        P2Wm = rt.tile([P, KT * S], f16, tag="P2Wm", name=f"P2Wm{ci}")
        nc.vector.tensor_tensor(P2Wm[:], P2[:], Wm[:], ALU.mult)

        # left features: mask on DVE (tiny), mask*u_k products on GPSIMD
        mask = lt.tile([P, KT * T], f16, tag="mask", name=f"mask{ci}")
        nc.vector.tensor_scalar(mask[:], cellf[:], float(ci - CH), None,
                                ALU.is_equal)
        ls = []
        for k4 in range(4):
            lk = lt.tile([P, KT * T], f16, tag=f"l{k4}", name=f"l{k4}_{ci}")
            nc.gpsimd.tensor_tensor(lk[:], mask[:], upow[k4][:], ALU.mult)
            ls.append(lk)

        last_cell = (ci == n_cells - 1)
        # reversed pair order: the final PE mms depend on the earliest-ready
        # right tiles (P), shrinking the tail
        pairs = [(ls[3], P2Wm), (ls[2], PWm), (ls[1], Wm), (ls[0], Pt)]
        for pi, (Lt, Gt) in enumerate(pairs):
            for k in range(KT):
                for c in range(NSC):
                    stop = (last_cell and pi == 3 and k == KT - 1
                            and c == NSC - 1)
                    nc.tensor.matmul(
                        scoresT[:, c * T:(c + 1) * T],
                        Gt[:, k * S + c * P:k * S + (c + 1) * P],
                        Lt[:, k * T:(k + 1) * T],
                        start=False, stop=stop)

    # ---- epilogue: transpose, softmax, c, attn_h
    scoresT_sb = pers.tile([P, NSC * T], f32, tag="scT_sb")
    nc.vector.tensor_copy(scoresT_sb[:], scoresT[:])
    if "scdbg" in dout:
        nc.sync.dma_start(out=dout["scdbg"][:], in_=scoresT_sb[:])
    scores = psOut.tile([T, S], f32, tag="out512")
    for c in range(NSC):
        nc.tensor.transpose(scores[:, c * P:(c + 1) * P],
                            scoresT_sb[:, c * T:(c + 1) * T], eye128[:])

    # softmax over s; |scores| is small so exp without max-shift is safe
    align_sb = pers.tile([T, S], f32, tag="align_sb")
    sums = pers.tile([T, 1], f32, tag="sums")
    nc.scalar.activation(align_sb[:], scores[:], AF.Exp, accum_out=sums[:])
    recips = pers.tile([T, 1], f32, tag="recips")
    nc.vector.reciprocal(recips[:], sums[:])
    nc.vector.tensor_scalar_mul(align_sb[:], align_sb[:], recips[:])
    nc.sync.dma_start(out=dout["align"][:], in_=align_sb[:])

    # alignT via PE transpose, fp16 copies for the c-matmul
    alignT = []
    for j in range(NSC):
        pt = psSm.tile([P, T], f32, tag="ps")
        nc.tensor.transpose(pt[:], align_sb[:, j * P:(j + 1) * P], eye64[:])
        at = pers.tile([P, T], f16, tag=f"alignT{j}")
        nc.vector.tensor_copy(at[:], pt[:])
        alignT.append(at)

    # cT[h, t] = sum_s enc_t[s,h] * alignT[s,t]
    cT = []
    for m in range(KT):
        pc = psSm.tile([P, T], f32, tag="ps")
        for j in range(NSC):
            nc.tensor.matmul(pc[:], enc_t[:, j * H + m * P:j * H + (m + 1) * P],
                             alignT[j][:], start=(j == 0), stop=(j == NSC - 1))
        ct = pers.tile([P, T], f16, tag=f"cT{m}")
        nc.vector.tensor_copy(ct[:], pc[:])
        cT.append(ct)

    # attn_h = [c, dec] @ Wo + bo  (dec part + bias accumulated early above)
    for k in range(KT):
        nc.tensor.matmul(pa[:], cT[k][:], wo16[:, k * H:(k + 1) * H],
                         start=False, stop=(k == KT - 1))
    attn_sb = pers.tile([T, H], f32, tag="attn_sb")
    nc.vector.tensor_copy(attn_sb[:], pa[:])
    nc.sync.dma_start(out=dout["attn_h"][:], in_=attn_sb[:])


def build(debug_scores=False):
    key = ("cells", debug_scores, C_CELLS, DELTA)
    if key in _BUILT:
        return _BUILT[key]
    from contextlib import ExitStack

    import concourse.bacc as bacc
    import concourse.mybir as mybir
    import concourse.tile as tile

    f32 = mybir.dt.float32
    f16 = mybir.dt.float16
    nc = bacc.Bacc("TRN2", target_bir_lowering=False, debug=False)
    in_specs = [
        ("cov16", [1, S], f16), ("wcov16", [1, H], f16),
        ("encT16", [P, KT * S], f16), ("wc16", [P, KT * H], f16),
        ("decT16", [P, KT * T], f16), ("wq16", [P, KT * H], f16),
        ("bq16", [1, H], f16), ("vrep16", [P, KT * T], f16),
        ("enc16", [P, KT * H], f16), ("wo16", [P, 2 * KT * H], f16),
        ("bo16", [1, H], f16), ("eye64", [T, T], f32), ("eye128", [P, P], f32),
    ]
    out_specs = [("attn_h", [T, H], f32), ("align", [T, S], f32)]
    if debug_scores:
        out_specs.append(("scdbg", [P, NSC * T], f32))
    din = {n: nc.declare_dram_parameter(n, s, d, isOutput=False)
           for n, s, d in in_specs}
    dout = {n: nc.declare_dram_parameter(n, s, d, isOutput=True)
            for n, s, d in out_specs}
    with ExitStack() as ctx:
        tc = ctx.enter_context(tile.TileContext(nc))
        _emit(nc, tc, ctx, din, dout)
    nc.compile()
    _BUILT[key] = nc
    return nc


def _merge(x, chunks):
    """[chunks*P, F] -> [P, chunks*F] fp16 (partition-major merge)."""
    cp, F = x.shape
    assert cp == chunks * P
    return np.ascontiguousarray(
        x.reshape(chunks, P, F).transpose(1, 0, 2).reshape(P, chunks * F)
    ).astype(np.float16)


def prep_core_inputs(inputs):
    """Host-side shard: per-core input dicts (core b <- batch element b)."""
    dec = np.asarray(inputs["attn_dec_state"], np.float32)   # [T,B,H]
    encr = np.asarray(inputs["attn_enc_state"], np.float32)  # [S,B,H]
    cov = np.asarray(inputs["attn_coverage"], np.float32)    # [B,S]
    Wq = np.asarray(inputs["Wq"], np.float32)
    Wc = np.asarray(inputs["Wc"], np.float32)
    Wo = np.asarray(inputs["Wo"], np.float32)
    v = np.asarray(inputs["v"], np.float32)
    bq = np.asarray(inputs["bq"], np.float32)[None, :]
    bo = np.asarray(inputs["bo"], np.float32)[None, :]
    wcov = np.asarray(inputs["wcov"], np.float32)[None, :]
    vrep = np.zeros((P, KT * T), np.float32)
    for k in range(KT):
        vrep[:, k * T:(k + 1) * T] = v[k * P:(k + 1) * P][:, None]
    shared = dict(
        wq16=_merge(Wq, KT), wc16=_merge(Wc, KT), wo16=_merge(Wo, 2 * KT),
        vrep16=vrep.astype(np.float16), wcov16=wcov.astype(np.float16),
        bq16=bq.astype(np.float16), bo16=bo.astype(np.float16),
        eye64=np.eye(T, dtype=np.float32), eye128=np.eye(P, dtype=np.float32),
    )
    maps = []
    for b in range(B):
        e = np.ascontiguousarray(encr[:, b, :])           # [S,H]
        maps.append(dict(
            decT16=_merge(np.ascontiguousarray(dec[:, b, :].T), KT),
            enc16=_merge(e, NSC),
            encT16=_merge(np.ascontiguousarray(e.T), KT),
            cov16=np.ascontiguousarray(cov[b][None, :]).astype(np.float16),
            **shared,
        ))
    return maps


def kernel(**inputs):
    global LAST_RESULT
    nc = build()
    in_maps = prep_core_inputs(inputs)
    from concourse.bass_utils import run_bass_kernel_spmd

    trace = os.environ.get("ATTN_TRACE", "0") == "1"
    res = run_bass_kernel_spmd(nc, in_maps, list(range(B)), trace=trace)
    LAST_RESULT = res
    attn_h = np.stack([res.results[i]["attn_h"] for i in range(B)], axis=1)
    align = np.stack([res.results[i]["align"] for i in range(B)], axis=1)
    return attn_h, align


# revision 25
# speedup vs baseline: 1.1128x; 1.1128x over previous
"""Trainium2 Bass kernel: Bahdanau (additive) attention with coverage.

Reference computation (per batch element b, data-parallel over B=8 cores):
    enc   = tanh(enc_raw + cov[:,None]*wcov)            [S,H]
    a1    = dec @ Wq + bq                               [T,H]
    a2    = enc @ Wc                                    [S,H]
    scores[t,s] = sum_h v[h] * tanh(a1[t,h] + a2[s,h])  [T,S]
    align = softmax(scores, -1)                         [T,S]
    c     = align @ enc                                 [T,H]
    attn_h = [c, dec] @ Wo + bo                         [T,H]
Outputs: attn_h -> [T,B,H], align -> [T,B,S].

Device strategy: cell-factorized tanh. Quantize x = a1 onto C=9 centers
(spacing DELTA), tau = tanh(x - ctr), P = tanh(y + ctr); then exactly
    tanh(x+y) = P + tau*W - tau^2*P*W + tau^3*P^2*W - ...   (W = 1-P^2)
truncated at tau^3 (max |tau| ~ tanh(DELTA/2) -> err ~5e-3 on scores,
align rel err ~5e-3 end-to-end, tolerance 2e-2). Each (cell, k) term is a
rank-1-in-h product of a LEFT tile (mask*v*tau^k over [h,t]) and a RIGHT
tile (P-polynomial over [h,s]), contracted over h by PE into transposed
scoresT[s,t] (out free = 64). ACT does 2 passes/cell (Tanh, Square),
DVE 3 products/cell + small left chains, GPSIMD does coverage adds and
PSUM->SBUF copies. All feature tiles fp16 (DVE 2x/4x modes, 1-cyc PE).
"""

import os

import numpy as np

T, B, S, H = 64, 8, 512, 512
P = 128
KT = H // P   # 4 partition chunks of H
NSC = S // P  # 4 partition chunks of S

C_CELLS = int(os.environ.get("ATTN_CELLS", "9"))
DELTA = float(os.environ.get("ATTN_DELTA", "1.0"))
MAGIC = float(1.5 * 2 ** 23)  # fp32 round-to-nearest-int via add/sub

_BUILT = {}
LAST_RESULT = None


def _emit(nc, tc, ctx, din, dout):
    import concourse.mybir as mybir

    f32 = mybir.dt.float32
    f16 = mybir.dt.float16
    AF = mybir.ActivationFunctionType
    ALU = mybir.AluOpType

    pers = ctx.enter_context(tc.tile_pool(name="pers", bufs=1))
    rt = ctx.enter_context(tc.tile_pool(name="rt", bufs=3))    # right tiles
    lt = ctx.enter_context(tc.tile_pool(name="lt", bufs=3))    # left tiles
    psT = ctx.enter_context(tc.tile_pool(name="psT", bufs=2, space="PSUM"))
    psSm = ctx.enter_context(tc.tile_pool(name="psSm", bufs=2, space="PSUM"))
    psOut = ctx.enter_context(tc.tile_pool(name="psOut", bufs=1, space="PSUM"))

    def ld(name, shape, dt):
        t = pers.tile(shape, dt, tag=name)
        nc.sync.dma_start(out=t[:], in_=din[name][:])
        return t

    # DMA order = need order.
    covr16 = ld("cov16", [1, S], f16)
    wcovr16 = ld("wcov16", [1, H], f16)
    encT16 = ld("encT16", [P, KT * S], f16)
    wc16 = ld("wc16", [P, KT * H], f16)
    decT16 = ld("decT16", [P, KT * T], f16)
    wq16 = ld("wq16", [P, KT * H], f16)
    bqr16 = ld("bq16", [1, H], f16)
    vrep16 = ld("vrep16", [P, KT * T], f16)
    enc16 = ld("enc16", [P, KT * H], f16)
    wo16 = ld("wo16", [P, 2 * KT * H], f16)
    bor16 = ld("bo16", [1, H], f16)
    eye64 = ld("eye64", [T, T], f32)
    eye128 = ld("eye128", [P, P], f32)
    ones16 = pers.tile([1, T], f16, tag="ones16")
    nc.vector.memset(ones16[:], 1.0)
    ones128 = pers.tile([1, P], f16, tag="ones128")
    nc.vector.memset(ones128[:], 1.0)
    # PE p-state warmup: ~3us of dependency-free junk matmuls so the real
    # prologue matmuls run at full clock
    warm = psT.tile([T, T], f32, tag="pt")
    for _ in range(18):
        nc.tensor.matmul(warm[:], ones16[0:1, :], ones16[0:1, :],
                         start=True, stop=True)

    # coverage in [H,S] layout: encT_t = tanh(encT + wcov (x) cov)
    # (adds on GPSIMD to keep DVE free)
    encT_t = pers.tile([P, KT * S], f16, tag="encT_t")
    for i in range(KT):
        op = psT.tile([P, S], f32, tag="pt")
        nc.tensor.matmul(op[:], wcovr16[0:1, i * P:(i + 1) * P], covr16[0:1, :],
                         start=True, stop=True)
        nc.vector.tensor_add(encT16[:, i * S:(i + 1) * S],
                             encT16[:, i * S:(i + 1) * S], op[:])
        nc.scalar.activation(encT_t[:, i * S:(i + 1) * S],
                             encT16[:, i * S:(i + 1) * S], AF.Tanh)

    # a2T[hout, (k,s)] merged fp16 tile
    a2T = pers.tile([P, KT * S], f16, tag="a2T")
    for m in range(KT):
        pm2 = psT.tile([P, S], f32, tag="pt")
        for k in range(KT):
            nc.tensor.matmul(pm2[:], wc16[:, k * H + m * P:k * H + (m + 1) * P],
                             encT_t[:, k * S:(k + 1) * S],
                             start=(k == 0), stop=(k == KT - 1))
        nc.vector.tensor_copy(a2T[:, m * S:(m + 1) * S], pm2[:])

    # a1T[hout, (k,t)] f32 (feeds cell quantization)
    a1T = pers.tile([P, KT * T], f32, tag="a1T")
    for m in range(KT):
        pm1 = psSm.tile([P, T], f32, tag="ps")
        for k in range(KT):
            nc.tensor.matmul(pm1[:], wq16[:, k * H + m * P:k * H + (m + 1) * P],
                             decT16[:, k * T:(k + 1) * T],
                             start=(k == 0), stop=False)
        nc.tensor.matmul(pm1[:], bqr16[0:1, m * P:(m + 1) * P], ones16[0:1, :],
                         start=False, stop=True)
        nc.vector.tensor_copy(a1T[:, m * T:(m + 1) * T], pm1[:])

    # coverage in [S,H] layout (for the c-matmul): enc_t = tanh(enc + cov (x) wcov)
    enc_t = pers.tile([P, KT * H], f16, tag="enc_t")
    for j in range(NSC):
        op = psT.tile([P, H], f32, tag="pt")
        nc.tensor.matmul(op[:], covr16[0:1, j * P:(j + 1) * P], wcovr16[0:1, :],
                         start=True, stop=True)
        nc.vector.tensor_add(enc16[:, j * H:(j + 1) * H],
                             enc16[:, j * H:(j + 1) * H], op[:])
        nc.scalar.activation(enc_t[:, j * H:(j + 1) * H],
                             enc16[:, j * H:(j + 1) * H], AF.Tanh)

    # attn_h dec-part (independent of the attention loop): start pa early
    pa = psOut.tile([T, H], f32, tag="pa")
    for k in range(KT):
        nc.tensor.matmul(pa[:], decT16[:, k * T:(k + 1) * T],
                         wo16[:, (KT + k) * H:(KT + k + 1) * H],
                         start=(k == 0), stop=False)
    nc.tensor.matmul(pa[:], ones16[0:1, :], bor16[0:1, :], start=False,
                     stop=False)

    # ---- left-side quantization: cellf = clamp(round(a1/DELTA)), taum = tanh(ctr-a1)
    CH = (C_CELLS - 1) // 2  # centers at DELTA*(-CH..CH)
    if DELTA != 1.0:
        r1 = pers.tile([P, KT * T], f32, tag="r1")
        nc.vector.tensor_scalar(r1[:], a1T[:], float(1.0 / DELTA), None, ALU.mult)
    else:
        r1 = a1T
    cellf = pers.tile([P, KT * T], f32, tag="cellf")
    nc.vector.tensor_scalar(cellf[:], r1[:], MAGIC, MAGIC, ALU.add, ALU.subtract)
    nc.vector.tensor_scalar(cellf[:], cellf[:], float(CH), float(-CH),
                            ALU.min, ALU.max)
    negd = pers.tile([P, KT * T], f32, tag="negd")
    nc.vector.scalar_tensor_tensor(negd[:], cellf[:], float(DELTA), a1T[:],
                                   ALU.mult, ALU.subtract)  # ctr - a1 = -delta
    taum = pers.tile([P, KT * T], f16, tag="taum")
    nc.scalar.activation(taum[:], negd[:], AF.Tanh)
    # global left power tiles u_k = vrep * taum^k (small DVE ops)
    u0 = vrep16
    u1 = pers.tile([P, KT * T], f16, tag="u1")
    nc.vector.tensor_tensor(u1[:], u0[:], taum[:], ALU.mult)
    u2 = pers.tile([P, KT * T], f16, tag="u2")
    nc.vector.tensor_tensor(u2[:], u1[:], taum[:], ALU.mult)
    u3 = pers.tile([P, KT * T], f16, tag="u3")
    nc.vector.tensor_tensor(u3[:], u2[:], taum[:], ALU.mult)
    upow = [u0, u1, u2, u3]

    # ---- scoresT accumulator; zero the bank once
    scoresT = psOut.tile([P, NSC * T], f32, tag="scT")
    zrow = pers.tile([1, NSC * T], f16, tag="zrow")
    nc.vector.memset(zrow[:], 0.0)
    nc.tensor.matmul(scoresT[:], ones128[0:1, :], zrow[0:1, :],
                     start=True, stop=False)

    # ---- per-cell features + PE contraction
    # series: tanh(x+y) = P + tau*W - tau^2 P W + tau^3 P^2 W   (W = 1-P^2)
    # with taum = -tau and Wm = P^2-1 = -W the pairs are:
    #   (mv, P), (mv*taum, Wm), (mv*taum^2, P*Wm), (mv*taum^3, P^2*Wm)
    n_cells = C_CELLS
    ctrb = pers.tile([P, n_cells], f32, tag="ctrb")
    for ci in range(n_cells):
        nc.vector.memset(ctrb[:, ci:ci + 1], float(DELTA * (ci - CH)))
    for ci in range(n_cells):
        Pt = rt.tile([P, KT * S], f16, tag="P", name=f"P{ci}")
        nc.scalar.activation(Pt[:], a2T[:], AF.Tanh, bias=ctrb[:, ci:ci + 1])
        P2 = rt.tile([P, KT * S], f16, tag="P2", name=f"P2{ci}")
        nc.scalar.activation(P2[:], Pt[:], AF.Square)
        Wm = rt.tile([P, KT * S], f16, tag="Wm", name=f"Wm{ci}")
        nc.vector.tensor_scalar(Wm[:], P2[:], 1.0, 1.0, ALU.mult, ALU.subtract)
        PWm = rt.tile([P, KT * S], f16, tag="PWm", name=f"PWm{ci}")
        nc.vector.tensor_tensor(PWm[:], Pt[:], Wm[:], ALU.mult)
        P2Wm = rt.tile([P, KT * S], f16, tag="P2Wm", name=f"P2Wm{ci}")
        nc.vector.tensor_tensor(P2Wm[:], P2[:], Wm[:], ALU.mult)

        # left features: mask on DVE (tiny), mask*u_k products on GPSIMD
        mask = lt.tile([P, KT * T], f16, tag="mask", name=f"mask{ci}")
        nc.vector.tensor_scalar(mask[:], cellf[:], float(ci - CH), None,
                                ALU.is_equal)
        ls = []
        for k4 in range(4):
            lk = lt.tile([P, KT * T], f16, tag=f"l{k4}", name=f"l{k4}_{ci}")
            nc.gpsimd.tensor_tensor(lk[:], mask[:], upow[k4][:], ALU.mult)
            ls.append(lk)

        last_cell = (ci == n_cells - 1)
        pairs = [(ls[0], Pt), (ls[1], Wm), (ls[2], PWm), (ls[3], P2Wm)]
        for pi, (Lt, Gt) in enumerate(pairs):
            for k in range(KT):
                for c in range(NSC):
                    stop = (last_cell and pi == 3 and k == KT - 1
                            and c == NSC - 1)
                    nc.tensor.matmul(
                        scoresT[:, c * T:(c + 1) * T],
                        Gt[:, k * S + c * P:k * S + (c + 1) * P],
                        Lt[:, k * T:(k + 1) * T],
                        start=False, stop=stop)

    # ---- epilogue: transpose, softmax, c, attn_h
    scoresT_sb = pers.tile([P, NSC * T], f32, tag="scT_sb")
    nc.vector.tensor_copy(scoresT_sb[:], scoresT[:])
    if "scdbg" in dout:
        nc.sync.dma_start(out=dout["scdbg"][:], in_=scoresT_sb[:])
    scores = psOut.tile([T, S], f32, tag="out512")
    for c in range(NSC):
        nc.tensor.transpose(scores[:, c * P:(c + 1) * P],
                            scoresT_sb[:, c * T:(c + 1) * T], eye128[:])

    # softmax over s; |scores| is small so exp without max-shift is safe
    align_sb = pers.tile([T, S], f32, tag="align_sb")
    sums = pers.tile([T, 1], f32, tag="sums")
    nc.scalar.activation(align_sb[:], scores[:], AF.Exp, accum_out=sums[:])
    recips = pers.tile([T, 1], f32, tag="recips")
    nc.vector.reciprocal(recips[:], sums[:])
    nc.vector.tensor_scalar_mul(align_sb[:], align_sb[:], recips[:])
    nc.sync.dma_start(out=dout["align"][:], in_=align_sb[:])

    # alignT via PE transpose, fp16 copies for the c-matmul
    alignT = []
    for j in range(NSC):
        pt = psSm.tile([P, T], f32, tag="ps")
        nc.tensor.transpose(pt[:], align_sb[:, j * P:(j + 1) * P], eye64[:])
        at = pers.tile([P, T], f16, tag=f"alignT{j}")
        nc.vector.tensor_copy(at[:], pt[:])
        alignT.append(at)

    # cT[h, t] = sum_s enc_t[s,h] * alignT[s,t]
    cT = []
    for m in range(KT):
        pc = psSm.tile([P, T], f32, tag="ps")
        for j in range(NSC):
            nc.tensor.matmul(pc[:], enc_t[:, j * H + m * P:j * H + (m + 1) * P],
                             alignT[j][:], start=(j == 0), stop=(j == NSC - 1))
        ct = pers.tile([P, T], f16, tag=f"cT{m}")
        nc.vector.tensor_copy(ct[:], pc[:])
        cT.append(ct)

    # attn_h = [c, dec] @ Wo + bo  (dec part + bias accumulated early above)
    for k in range(KT):
        nc.tensor.matmul(pa[:], cT[k][:], wo16[:, k * H:(k + 1) * H],
                         start=False, stop=(k == KT - 1))
    attn_sb = pers.tile([T, H], f32, tag="attn_sb")
    nc.vector.tensor_copy(attn_sb[:], pa[:])
    nc.sync.dma_start(out=dout["attn_h"][:], in_=attn_sb[:])


def build(debug_scores=False):
    key = ("cells", debug_scores, C_CELLS, DELTA)
    if key in _BUILT:
        return _BUILT[key]
    from contextlib import ExitStack

    import concourse.bacc as bacc
    import concourse.mybir as mybir
    import concourse.tile as tile

    f32 = mybir.dt.float32
    f16 = mybir.dt.float16
    nc = bacc.Bacc("TRN2", target_bir_lowering=False, debug=False)
    in_specs = [
        ("cov16", [1, S], f16), ("wcov16", [1, H], f16),
        ("encT16", [P, KT * S], f16), ("wc16", [P, KT * H], f16),
        ("decT16", [P, KT * T], f16), ("wq16", [P, KT * H], f16),
        ("bq16", [1, H], f16), ("vrep16", [P, KT * T], f16),
        ("enc16", [P, KT * H], f16), ("wo16", [P, 2 * KT * H], f16),
        ("bo16", [1, H], f16), ("eye64", [T, T], f32), ("eye128", [P, P], f32),
    ]
    out_specs = [("attn_h", [T, H], f32), ("align", [T, S], f32)]
    if debug_scores:
        out_specs.append(("scdbg", [P, NSC * T], f32))
    din = {n: nc.declare_dram_parameter(n, s, d, isOutput=False)
           for n, s, d in in_specs}
    dout = {n: nc.declare_dram_parameter(n, s, d, isOutput=True)
            for n, s, d in out_specs}
    with ExitStack() as ctx:
        tc = ctx.enter_context(tile.TileContext(nc))
        _emit(nc, tc, ctx, din, dout)
    nc.compile()
    _BUILT[key] = nc
    return nc


def _merge(x, chunks):
    """[chunks*P, F] -> [P, chunks*F] fp16 (partition-major merge)."""
    cp, F = x.shape
    assert cp == chunks * P
    return np.ascontiguousarray(
        x.reshape(chunks, P, F).transpose(1, 0, 2).reshape(P, chunks * F)
    ).astype(np.float16)


def prep_core_inputs(inputs):
    """Host-side shard: per-core input dicts (core b <- batch element b)."""
    dec = np.asarray(inputs["attn_dec_state"], np.float32)   # [T,B,H]
    encr = np.asarray(inputs["attn_enc_state"], np.float32)  # [S,B,H]
    cov = np.asarray(inputs["attn_coverage"], np.float32)    # [B,S]
    Wq = np.asarray(inputs["Wq"], np.float32)
    Wc = np.asarray(inputs["Wc"], np.float32)
    Wo = np.asarray(inputs["Wo"], np.float32)
    v = np.asarray(inputs["v"], np.float32)
    bq = np.asarray(inputs["bq"], np.float32)[None, :]
    bo = np.asarray(inputs["bo"], np.float32)[None, :]
    wcov = np.asarray(inputs["wcov"], np.float32)[None, :]
    vrep = np.zeros((P, KT * T), np.float32)
    for k in range(KT):
        vrep[:, k * T:(k + 1) * T] = v[k * P:(k + 1) * P][:, None]
    shared = dict(
        wq16=_merge(Wq, KT), wc16=_merge(Wc, KT), wo16=_merge(Wo, 2 * KT),
        vrep16=vrep.astype(np.float16), wcov16=wcov.astype(np.float16),
        bq16=bq.astype(np.float16), bo16=bo.astype(np.float16),
        eye64=np.eye(T, dtype=np.float32), eye128=np.eye(P, dtype=np.float32),
    )
    maps = []
    for b in range(B):
        e = np.ascontiguousarray(encr[:, b, :])           # [S,H]
        maps.append(dict(
            decT16=_merge(np.ascontiguousarray(dec[:, b, :].T), KT),
            enc16=_merge(e, NSC),
            encT16=_merge(np.ascontiguousarray(e.T), KT),
            cov16=np.ascontiguousarray(cov[b][None, :]).astype(np.float16),
            **shared,
        ))
    return maps


def kernel(**inputs):
    global LAST_RESULT
    nc = build()
    in_maps = prep_core_inputs(inputs)
    from concourse.bass_utils import run_bass_kernel_spmd

    trace = os.environ.get("ATTN_TRACE", "0") == "1"
    res = run_bass_kernel_spmd(nc, in_maps, list(range(B)), trace=trace)
    LAST_RESULT = res
    attn_h = np.stack([res.results[i]["attn_h"] for i in range(B)], axis=1)
    align = np.stack([res.results[i]["align"] for i in range(B)], axis=1)
    return attn_h, align


# revision 28
# speedup vs baseline: 1.1343x; 1.0193x over previous
"""Trainium2 Bass kernel: Bahdanau (additive) attention with coverage.

Reference computation (per batch element b, data-parallel over B=8 cores):
    enc   = tanh(enc_raw + cov[:,None]*wcov)            [S,H]
    a1    = dec @ Wq + bq                               [T,H]
    a2    = enc @ Wc                                    [S,H]
    scores[t,s] = sum_h v[h] * tanh(a1[t,h] + a2[s,h])  [T,S]
    align = softmax(scores, -1)                         [T,S]
    c     = align @ enc                                 [T,H]
    attn_h = [c, dec] @ Wo + bo                         [T,H]
Outputs: attn_h -> [T,B,H], align -> [T,B,S].

Device strategy: cell-factorized tanh. Quantize x = a1 onto C=9 centers
(spacing DELTA), tau = tanh(x - ctr), P = tanh(y + ctr); then exactly
    tanh(x+y) = P + tau*W - tau^2*P*W + tau^3*P^2*W - ...   (W = 1-P^2)
truncated at tau^3 (max |tau| ~ tanh(DELTA/2) -> err ~5e-3 on scores,
align rel err ~5e-3 end-to-end, tolerance 2e-2). Each (cell, k) term is a
rank-1-in-h product of a LEFT tile (mask*v*tau^k over [h,t]) and a RIGHT
tile (P-polynomial over [h,s]), contracted over h by PE into transposed
scoresT[s,t] (out free = 64). ACT does 2 passes/cell (Tanh, Square),
DVE 3 products/cell + small left chains, GPSIMD does coverage adds and
PSUM->SBUF copies. All feature tiles fp16 (DVE 2x/4x modes, 1-cyc PE).
"""

import os

import numpy as np

T, B, S, H = 64, 8, 512, 512
P = 128
KT = H // P   # 4 partition chunks of H
NSC = S // P  # 4 partition chunks of S

C_CELLS = int(os.environ.get("ATTN_CELLS", "9"))
DELTA = float(os.environ.get("ATTN_DELTA", "1.0"))
MAGIC = float(1.5 * 2 ** 23)  # fp32 round-to-nearest-int via add/sub

_BUILT = {}
LAST_RESULT = None


def _emit(nc, tc, ctx, din, dout):
    import concourse.mybir as mybir

    f32 = mybir.dt.float32
    f16 = mybir.dt.float16
    AF = mybir.ActivationFunctionType
    ALU = mybir.AluOpType

    pers = ctx.enter_context(tc.tile_pool(name="pers", bufs=1))
    rt = ctx.enter_context(tc.tile_pool(name="rt", bufs=3))    # right tiles
    lt = ctx.enter_context(tc.tile_pool(name="lt", bufs=3))    # left tiles
    psT = ctx.enter_context(tc.tile_pool(name="psT", bufs=2, space="PSUM"))
    psSm = ctx.enter_context(tc.tile_pool(name="psSm", bufs=2, space="PSUM"))
    psOut = ctx.enter_context(tc.tile_pool(name="psOut", bufs=1, space="PSUM"))

    def ld(name, shape, dt):
        t = pers.tile(shape, dt, tag=name)
        nc.sync.dma_start(out=t[:], in_=din[name][:])
        return t

    # DMA order = need order.
    covr16 = ld("cov16", [1, S], f16)
    wcovr16 = ld("wcov16", [1, H], f16)
    encT16 = ld("encT16", [P, KT * S], f16)
    wc16 = ld("wc16", [P, KT * H], f16)
    decT16 = ld("decT16", [P, KT * T], f16)
    wq16 = ld("wq16", [P, KT * H], f16)
    bqr16 = ld("bq16", [1, H], f16)
    vrep16 = ld("vrep16", [P, KT * T], f16)
    enc16 = ld("enc16", [P, KT * H], f16)
    wo16 = ld("wo16", [P, 2 * KT * H], f16)
    bor16 = ld("bo16", [1, H], f16)
    eye64 = ld("eye64", [T, T], f32)
    eye128 = ld("eye128", [P, P], f32)
    ones16 = pers.tile([1, T], f16, tag="ones16")
    nc.vector.memset(ones16[:], 1.0)
    ones128 = pers.tile([1, P], f16, tag="ones128")
    nc.vector.memset(ones128[:], 1.0)
    # PE p-state warmup: ~3us of dependency-free junk matmuls so the real
    # prologue matmuls run at full clock
    warm = psT.tile([T, T], f32, tag="pt")
    for _ in range(18):
        nc.tensor.matmul(warm[:], ones16[0:1, :], ones16[0:1, :],
                         start=True, stop=True)

    # coverage in [H,S] layout: encT_t = tanh(encT + wcov (x) cov)
    # (adds on GPSIMD to keep DVE free)
    encT_t = pers.tile([P, KT * S], f16, tag="encT_t")
    for i in range(KT):
        op = psT.tile([P, S], f32, tag="pt")
        nc.tensor.matmul(op[:], wcovr16[0:1, i * P:(i + 1) * P], covr16[0:1, :],
                         start=True, stop=True)
        nc.vector.tensor_add(encT16[:, i * S:(i + 1) * S],
                             encT16[:, i * S:(i + 1) * S], op[:])
        nc.scalar.activation(encT_t[:, i * S:(i + 1) * S],
                             encT16[:, i * S:(i + 1) * S], AF.Tanh)

    # a1T[hout, (k,t)] f32 (feeds cell quantization)
    a1T = pers.tile([P, KT * T], f32, tag="a1T")
    for m in range(KT):
        pm1 = psSm.tile([P, T], f32, tag="ps")
        for k in range(KT):
            nc.tensor.matmul(pm1[:], wq16[:, k * H + m * P:k * H + (m + 1) * P],
                             decT16[:, k * T:(k + 1) * T],
                             start=(k == 0), stop=False)
        nc.tensor.matmul(pm1[:], bqr16[0:1, m * P:(m + 1) * P], ones16[0:1, :],
                         start=False, stop=True)
        nc.vector.tensor_copy(a1T[:, m * T:(m + 1) * T], pm1[:])

    # ---- left-side quantization: cellf = clamp(round(a1/DELTA)), taum = tanh(ctr-a1)
    CH = (C_CELLS - 1) // 2  # centers at DELTA*(-CH..CH)
    if DELTA != 1.0:
        r1 = pers.tile([P, KT * T], f32, tag="r1")
        nc.vector.tensor_scalar(r1[:], a1T[:], float(1.0 / DELTA), None, ALU.mult)
    else:
        r1 = a1T
    cellf = pers.tile([P, KT * T], f32, tag="cellf")
    nc.vector.tensor_scalar(cellf[:], r1[:], MAGIC, MAGIC, ALU.add, ALU.subtract)
    nc.vector.tensor_scalar(cellf[:], cellf[:], float(CH), float(-CH),
                            ALU.min, ALU.max)
    negd = pers.tile([P, KT * T], f32, tag="negd")
    nc.vector.scalar_tensor_tensor(negd[:], cellf[:], float(DELTA), a1T[:],
                                   ALU.mult, ALU.subtract)  # ctr - a1 = -delta
    taum = pers.tile([P, KT * T], f16, tag="taum")
    nc.scalar.activation(taum[:], negd[:], AF.Tanh)
    # global left power tiles u_k = vrep * taum^k (small DVE ops)
    u0 = vrep16
    u1 = pers.tile([P, KT * T], f16, tag="u1")
    nc.vector.tensor_tensor(u1[:], u0[:], taum[:], ALU.mult)
    u2 = pers.tile([P, KT * T], f16, tag="u2")
    nc.vector.tensor_tensor(u2[:], u1[:], taum[:], ALU.mult)
    u3 = pers.tile([P, KT * T], f16, tag="u3")
    nc.vector.tensor_tensor(u3[:], u2[:], taum[:], ALU.mult)
    upow = [u0, u1, u2, u3]

    # a2T[hout, (k,s)] merged fp16 tile
    a2T = pers.tile([P, KT * S], f16, tag="a2T")
    for m in range(KT):
        pm2 = psT.tile([P, S], f32, tag="pt")
        for k in range(KT):
            nc.tensor.matmul(pm2[:], wc16[:, k * H + m * P:k * H + (m + 1) * P],
                             encT_t[:, k * S:(k + 1) * S],
                             start=(k == 0), stop=(k == KT - 1))
        nc.vector.tensor_copy(a2T[:, m * S:(m + 1) * S], pm2[:])

    # coverage in [S,H] layout (for the c-matmul): enc_t = tanh(enc + cov (x) wcov)
    enc_t = pers.tile([P, KT * H], f16, tag="enc_t")
    for j in range(NSC):
        op = psT.tile([P, H], f32, tag="pt")
        nc.tensor.matmul(op[:], covr16[0:1, j * P:(j + 1) * P], wcovr16[0:1, :],
                         start=True, stop=True)
        nc.vector.tensor_add(enc16[:, j * H:(j + 1) * H],
                             enc16[:, j * H:(j + 1) * H], op[:])
    nc.scalar.activation(enc_t[:], enc16[:], AF.Tanh)

    # attn_h dec-part (independent of the attention loop): start pa early
    pa = psOut.tile([T, H], f32, tag="pa")
    for k in range(KT):
        nc.tensor.matmul(pa[:], decT16[:, k * T:(k + 1) * T],
                         wo16[:, (KT + k) * H:(KT + k + 1) * H],
                         start=(k == 0), stop=False)
    nc.tensor.matmul(pa[:], ones16[0:1, :], bor16[0:1, :], start=False,
                     stop=False)


    # ---- scoresT accumulator; zero the bank once
    scoresT = psOut.tile([P, NSC * T], f32, tag="scT")
    zrow = pers.tile([1, NSC * T], f16, tag="zrow")
    nc.vector.memset(zrow[:], 0.0)
    nc.tensor.matmul(scoresT[:], ones128[0:1, :], zrow[0:1, :],
                     start=True, stop=False)

    # ---- per-cell features + PE contraction
    # series: tanh(x+y) = P + tau*W - tau^2 P W + tau^3 P^2 W   (W = 1-P^2)
    # with taum = -tau and Wm = P^2-1 = -W the pairs are:
    #   (mv, P), (mv*taum, Wm), (mv*taum^2, P*Wm), (mv*taum^3, P^2*Wm)
    n_cells = C_CELLS
    ctrb = pers.tile([P, n_cells], f32, tag="ctrb")
    for ci in range(n_cells):
        nc.vector.memset(ctrb[:, ci:ci + 1], float(DELTA * (ci - CH)))
    for ci in range(n_cells):
        Pt = rt.tile([P, KT * S], f16, tag="P", name=f"P{ci}")
        nc.scalar.activation(Pt[:], a2T[:], AF.Tanh, bias=ctrb[:, ci:ci + 1])
        P2 = rt.tile([P, KT * S], f16, tag="P2", name=f"P2{ci}")
        nc.scalar.activation(P2[:], Pt[:], AF.Square)
        Wm = rt.tile([P, KT * S], f16, tag="Wm", name=f"Wm{ci}")
        nc.vector.tensor_scalar(Wm[:], P2[:], 1.0, 1.0, ALU.mult, ALU.subtract)
        PWm = rt.tile([P, KT * S], f16, tag="PWm", name=f"PWm{ci}")
        nc.vector.tensor_tensor(PWm[:], Pt[:], Wm[:], ALU.mult)
        P2Wm = rt.tile([P, KT * S], f16, tag="P2Wm", name=f"P2Wm{ci}")
        nc.vector.tensor_tensor(P2Wm[:], P2[:], Wm[:], ALU.mult)

        # left features: mask on DVE (tiny), mask*u_k products on GPSIMD
        mask = lt.tile([P, KT * T], f16, tag="mask", name=f"mask{ci}")
        nc.vector.tensor_scalar(mask[:], cellf[:], float(ci - CH), None,
                                ALU.is_equal)
        ls = []
        for k4 in range(4):
            lk = lt.tile([P, KT * T], f16, tag=f"l{k4}", name=f"l{k4}_{ci}")
            nc.gpsimd.tensor_tensor(lk[:], mask[:], upow[k4][:], ALU.mult)
            ls.append(lk)

        last_cell = (ci == n_cells - 1)
        pairs = [(ls[0], Pt), (ls[1], Wm), (ls[2], PWm), (ls[3], P2Wm)]
        for pi, (Lt, Gt) in enumerate(pairs):
            for k in range(KT):
                for c in range(NSC):
                    stop = (last_cell and pi == 3 and k == KT - 1
                            and c == NSC - 1)
                    nc.tensor.matmul(
                        scoresT[:, c * T:(c + 1) * T],
                        Gt[:, k * S + c * P:k * S + (c + 1) * P],
                        Lt[:, k * T:(k + 1) * T],
                        start=False, stop=stop)

    # ---- epilogue: transpose, softmax, c, attn_h
    scoresT_sb = pers.tile([P, NSC * T], f32, tag="scT_sb")
    nc.vector.tensor_copy(scoresT_sb[:], scoresT[:])
    if "scdbg" in dout:
        nc.sync.dma_start(out=dout["scdbg"][:], in_=scoresT_sb[:])
    scores = psOut.tile([T, S], f32, tag="out512")
    for c in range(NSC):
        nc.tensor.transpose(scores[:, c * P:(c + 1) * P],
                            scoresT_sb[:, c * T:(c + 1) * T], eye128[:])

    # softmax over s; |scores| is small so exp without max-shift is safe
    align_sb = pers.tile([T, S], f32, tag="align_sb")
    sums = pers.tile([T, 1], f32, tag="sums")
    nc.scalar.activation(align_sb[:], scores[:], AF.Exp, accum_out=sums[:])
    recips = pers.tile([T, 1], f32, tag="recips")
    nc.vector.reciprocal(recips[:], sums[:])
    nc.vector.tensor_scalar_mul(align_sb[:], align_sb[:], recips[:])
    nc.sync.dma_start(out=dout["align"][:], in_=align_sb[:])

    # alignT via PE transpose, fp16 copies for the c-matmul
    alignT = []
    for j in range(NSC):
        pt = psSm.tile([P, T], f32, tag="ps")
        nc.tensor.transpose(pt[:], align_sb[:, j * P:(j + 1) * P], eye64[:])
        at = pers.tile([P, T], f16, tag=f"alignT{j}")
        nc.vector.tensor_copy(at[:], pt[:])
        alignT.append(at)

    # cT[h, t] = sum_s enc_t[s,h] * alignT[s,t]  (j outer: overlaps transposes)
    pc = psOut.tile([P, KT * T], f32, tag="pc")
    nc.tensor.matmul(pc[:], ones128[0:1, :], zrow[0:1, :], start=True, stop=False)
    for j in range(NSC):
        for m in range(KT):
            nc.tensor.matmul(pc[:, m * T:(m + 1) * T],
                             enc_t[:, j * H + m * P:j * H + (m + 1) * P],
                             alignT[j][:], start=False,
                             stop=(j == NSC - 1 and m == KT - 1))
    cT = []
    for m in range(KT):
        ct = pers.tile([P, T], f16, tag=f"cT{m}")
        nc.vector.tensor_copy(ct[:], pc[:, m * T:(m + 1) * T])
        cT.append(ct)

    # attn_h = [c, dec] @ Wo + bo  (dec part + bias accumulated early above)
    for k in range(KT):
        nc.tensor.matmul(pa[:], cT[k][:], wo16[:, k * H:(k + 1) * H],
                         start=False, stop=(k == KT - 1))
    attn_sb = pers.tile([T, H], f32, tag="attn_sb")
    nc.vector.tensor_copy(attn_sb[:], pa[:])
    nc.sync.dma_start(out=dout["attn_h"][:], in_=attn_sb[:])


def build(debug_scores=False):
    key = ("cells", debug_scores, C_CELLS, DELTA)
    if key in _BUILT:
        return _BUILT[key]
    from contextlib import ExitStack

    import concourse.bacc as bacc
    import concourse.mybir as mybir
    import concourse.tile as tile

    f32 = mybir.dt.float32
    f16 = mybir.dt.float16
    nc = bacc.Bacc("TRN2", target_bir_lowering=False, debug=False)
    in_specs = [
        ("cov16", [1, S], f16), ("wcov16", [1, H], f16),
        ("encT16", [P, KT * S], f16), ("wc16", [P, KT * H], f16),
        ("decT16", [P, KT * T], f16), ("wq16", [P, KT * H], f16),
        ("bq16", [1, H], f16), ("vrep16", [P, KT * T], f16),
        ("enc16", [P, KT * H], f16), ("wo16", [P, 2 * KT * H], f16),
        ("bo16", [1, H], f16), ("eye64", [T, T], f32), ("eye128", [P, P], f32),
    ]
    out_specs = [("attn_h", [T, H], f32), ("align", [T, S], f32)]
    if debug_scores:
        out_specs.append(("scdbg", [P, NSC * T], f32))
    din = {n: nc.declare_dram_parameter(n, s, d, isOutput=False)
           for n, s, d in in_specs}
    dout = {n: nc.declare_dram_parameter(n, s, d, isOutput=True)
            for n, s, d in out_specs}
    with ExitStack() as ctx:
        tc = ctx.enter_context(tile.TileContext(nc))
        _emit(nc, tc, ctx, din, dout)
    nc.compile()
    _BUILT[key] = nc
    return nc


def _merge(x, chunks):
    """[chunks*P, F] -> [P, chunks*F] fp16 (partition-major merge)."""
    cp, F = x.shape
    assert cp == chunks * P
    return np.ascontiguousarray(
        x.reshape(chunks, P, F).transpose(1, 0, 2).reshape(P, chunks * F)
    ).astype(np.float16)


def prep_core_inputs(inputs):
    """Host-side shard: per-core input dicts (core b <- batch element b)."""
    dec = np.asarray(inputs["attn_dec_state"], np.float32)   # [T,B,H]
    encr = np.asarray(inputs["attn_enc_state"], np.float32)  # [S,B,H]
    cov = np.asarray(inputs["attn_coverage"], np.float32)    # [B,S]
    Wq = np.asarray(inputs["Wq"], np.float32)
    Wc = np.asarray(inputs["Wc"], np.float32)
    Wo = np.asarray(inputs["Wo"], np.float32)
    v = np.asarray(inputs["v"], np.float32)
    bq = np.asarray(inputs["bq"], np.float32)[None, :]
    bo = np.asarray(inputs["bo"], np.float32)[None, :]
    wcov = np.asarray(inputs["wcov"], np.float32)[None, :]
    vrep = np.zeros((P, KT * T), np.float32)
    for k in range(KT):
        vrep[:, k * T:(k + 1) * T] = v[k * P:(k + 1) * P][:, None]
    shared = dict(
        wq16=_merge(Wq, KT), wc16=_merge(Wc, KT), wo16=_merge(Wo, 2 * KT),
        vrep16=vrep.astype(np.float16), wcov16=wcov.astype(np.float16),
        bq16=bq.astype(np.float16), bo16=bo.astype(np.float16),
        eye64=np.eye(T, dtype=np.float32), eye128=np.eye(P, dtype=np.float32),
    )
    maps = []
    for b in range(B):
        e = np.ascontiguousarray(encr[:, b, :])           # [S,H]
        maps.append(dict(
            decT16=_merge(np.ascontiguousarray(dec[:, b, :].T), KT),
            enc16=_merge(e, NSC),
            encT16=_merge(np.ascontiguousarray(e.T), KT),
            cov16=np.ascontiguousarray(cov[b][None, :]).astype(np.float16),
            **shared,
        ))
    return maps


def kernel(**inputs):
    global LAST_RESULT
    nc = build()
    in_maps = prep_core_inputs(inputs)
    from concourse.bass_utils import run_bass_kernel_spmd

    trace = os.environ.get("ATTN_TRACE", "0") == "1"
    res = run_bass_kernel_spmd(nc, in_maps, list(range(B)), trace=trace)
    LAST_RESULT = res
    attn_h = np.stack([res.results[i]["attn_h"] for i in range(B)], axis=1)
    align = np.stack([res.results[i]["align"] for i in range(B)], axis=1)
    return attn_h, align


# revision 29
# speedup vs baseline: 1.4051x; 1.2388x over previous
"""Trainium2 Bass kernel: Bahdanau (additive) attention with coverage.

Reference computation (per batch element b, data-parallel over B=8 cores):
    enc   = tanh(enc_raw + cov[:,None]*wcov)            [S,H]
    a1    = dec @ Wq + bq                               [T,H]
    a2    = enc @ Wc                                    [S,H]
    scores[t,s] = sum_h v[h] * tanh(a1[t,h] + a2[s,h])  [T,S]
    align = softmax(scores, -1)                         [T,S]
    c     = align @ enc                                 [T,H]
    attn_h = [c, dec] @ Wo + bo                         [T,H]
Outputs: attn_h -> [T,B,H], align -> [T,B,S].

Device strategy: cell-factorized tanh. Quantize x = a1 onto C=9 centers
(spacing DELTA), tau = tanh(x - ctr), P = tanh(y + ctr); then exactly
    tanh(x+y) = P + tau*W - tau^2*P*W + tau^3*P^2*W - ...   (W = 1-P^2)
truncated at tau^3 (max |tau| ~ tanh(DELTA/2) -> err ~5e-3 on scores,
align rel err ~5e-3 end-to-end, tolerance 2e-2). Each (cell, k) term is a
rank-1-in-h product of a LEFT tile (mask*v*tau^k over [h,t]) and a RIGHT
tile (P-polynomial over [h,s]), contracted over h by PE into transposed
scoresT[s,t] (out free = 64). ACT does 2 passes/cell (Tanh, Square),
DVE 3 products/cell + small left chains, GPSIMD does coverage adds and
PSUM->SBUF copies. All feature tiles fp16 (DVE 2x/4x modes, 1-cyc PE).
"""

import os

import numpy as np

T, B, S, H = 64, 8, 512, 512
P = 128
KT = H // P   # 4 partition chunks of H
NSC = S // P  # 4 partition chunks of S

C_CELLS = int(os.environ.get("ATTN_CELLS", "9"))
DELTA = float(os.environ.get("ATTN_DELTA", "1.0"))
MAGIC = float(1.5 * 2 ** 23)  # fp32 round-to-nearest-int via add/sub

_BUILT = {}
LAST_RESULT = None


def _emit(nc, tc, ctx, din, dout):
    import concourse.mybir as mybir

    f32 = mybir.dt.float32
    f16 = mybir.dt.float16
    AF = mybir.ActivationFunctionType
    ALU = mybir.AluOpType

    pers = ctx.enter_context(tc.tile_pool(name="pers", bufs=1))
    rt = ctx.enter_context(tc.tile_pool(name="rt", bufs=3))    # right tiles
    lt = ctx.enter_context(tc.tile_pool(name="lt", bufs=3))    # left tiles
    psT = ctx.enter_context(tc.tile_pool(name="psT", bufs=2, space="PSUM"))
    psSm = ctx.enter_context(tc.tile_pool(name="psSm", bufs=2, space="PSUM"))
    psOut = ctx.enter_context(tc.tile_pool(name="psOut", bufs=1, space="PSUM"))

    def ld(name, shape, dt):
        t = pers.tile(shape, dt, tag=name)
        nc.sync.dma_start(out=t[:], in_=din[name][:])
        return t

    # DMA order = need order.
    covr16 = ld("cov16", [1, S], f16)
    wcovr16 = ld("wcov16", [1, H], f16)
    encT16 = ld("encT16", [P, KT * S], f16)
    wc16 = ld("wc16", [P, KT * H], f16)
    decT16 = ld("decT16", [P, KT * T], f16)
    wq16 = ld("wq16", [P, KT * H], f16)
    bqr16 = ld("bq16", [1, H], f16)
    vrep16 = ld("vrep16", [P, KT * T], f16)
    enc16 = ld("enc16", [P, KT * H], f16)
    wo16 = ld("wo16", [P, 2 * KT * H], f16)
    bor16 = ld("bo16", [1, H], f16)
    eye64 = ld("eye64", [T, T], f32)
    eye128 = ld("eye128", [P, P], f32)
    ones16 = pers.tile([1, T], f16, tag="ones16")
    nc.vector.memset(ones16[:], 1.0)
    ones128 = pers.tile([1, P], f16, tag="ones128")
    nc.vector.memset(ones128[:], 1.0)
    # PE p-state warmup: ~3us of dependency-free junk matmuls so the real
    # prologue matmuls run at full clock
    warm = psT.tile([T, T], f32, tag="pt")
    for _ in range(18):
        nc.tensor.matmul(warm[:], ones16[0:1, :], ones16[0:1, :],
                         start=True, stop=True)

    # coverage in [H,S] layout: encT_t = tanh(encT + wcov (x) cov)
    # (adds on GPSIMD to keep DVE free)
    encT_t = pers.tile([P, KT * S], f16, tag="encT_t")
    for i in range(KT):
        op = psT.tile([P, S], f32, tag="pt")
        nc.tensor.matmul(op[:], wcovr16[0:1, i * P:(i + 1) * P], covr16[0:1, :],
                         start=True, stop=True)
        nc.vector.tensor_add(encT16[:, i * S:(i + 1) * S],
                             encT16[:, i * S:(i + 1) * S], op[:])
        nc.scalar.activation(encT_t[:, i * S:(i + 1) * S],
                             encT16[:, i * S:(i + 1) * S], AF.Tanh)

    # a1T[hout, (k,t)] f32 (feeds cell quantization)
    a1T = pers.tile([P, KT * T], f32, tag="a1T")
    for m in range(KT):
        pm1 = psSm.tile([P, T], f32, tag="ps")
        for k in range(KT):
            nc.tensor.matmul(pm1[:], wq16[:, k * H + m * P:k * H + (m + 1) * P],
                             decT16[:, k * T:(k + 1) * T],
                             start=(k == 0), stop=False)
        nc.tensor.matmul(pm1[:], bqr16[0:1, m * P:(m + 1) * P], ones16[0:1, :],
                         start=False, stop=True)
        nc.vector.tensor_copy(a1T[:, m * T:(m + 1) * T], pm1[:])

    # ---- left-side quantization: cellf = clamp(round(a1/DELTA)), taum = tanh(ctr-a1)
    CH = (C_CELLS - 1) // 2  # centers at DELTA*(-CH..CH)
    if DELTA != 1.0:
        r1 = pers.tile([P, KT * T], f32, tag="r1")
        nc.vector.tensor_scalar(r1[:], a1T[:], float(1.0 / DELTA), None, ALU.mult)
    else:
        r1 = a1T
    cellf = pers.tile([P, KT * T], f32, tag="cellf")
    nc.vector.tensor_scalar(cellf[:], r1[:], MAGIC, MAGIC, ALU.add, ALU.subtract)
    nc.vector.tensor_scalar(cellf[:], cellf[:], float(CH), float(-CH),
                            ALU.min, ALU.max)
    negd = pers.tile([P, KT * T], f32, tag="negd")
    nc.vector.scalar_tensor_tensor(negd[:], cellf[:], float(DELTA), a1T[:],
                                   ALU.mult, ALU.subtract)  # ctr - a1 = -delta
    taum = pers.tile([P, KT * T], f16, tag="taum")
    nc.scalar.activation(taum[:], negd[:], AF.Tanh)
    # global left combos for the raw P-power pairing (softmax-invariant
    # parts of the series dropped):
    #   tanh(x+y) ~ [inv] + (1-tau^2) P + (-tau+tau^3) P^2 + tau^2 P^3 - tau^3 P^4
    # in taum = -tau:  A = v(1-taum^2), B = v(taum-taum^3), C = v taum^2,
    #                  Dg = v taum^3
    vt1 = pers.tile([P, KT * T], f16, tag="vt1")
    nc.vector.tensor_tensor(vt1[:], vrep16[:], taum[:], ALU.mult)
    vt2 = pers.tile([P, KT * T], f16, tag="vt2")
    nc.vector.tensor_tensor(vt2[:], vt1[:], taum[:], ALU.mult)
    uA = pers.tile([P, KT * T], f16, tag="uA")
    nc.vector.tensor_tensor(uA[:], vrep16[:], vt2[:], ALU.subtract)
    uD = pers.tile([P, KT * T], f16, tag="uD")
    nc.vector.tensor_tensor(uD[:], vt2[:], taum[:], ALU.mult)
    uB = pers.tile([P, KT * T], f16, tag="uB")
    nc.vector.tensor_tensor(uB[:], vt1[:], uD[:], ALU.subtract)
    upow = [uA, uB, vt2, uD]

    # a2T[hout, (k,s)] merged fp16 tile
    a2T = pers.tile([P, KT * S], f16, tag="a2T")
    for m in range(KT):
        pm2 = psT.tile([P, S], f32, tag="pt")
        for k in range(KT):
            nc.tensor.matmul(pm2[:], wc16[:, k * H + m * P:k * H + (m + 1) * P],
                             encT_t[:, k * S:(k + 1) * S],
                             start=(k == 0), stop=(k == KT - 1))
        nc.vector.tensor_copy(a2T[:, m * S:(m + 1) * S], pm2[:])

    # coverage in [S,H] layout (for the c-matmul): enc_t = tanh(enc + cov (x) wcov)
    enc_t = pers.tile([P, KT * H], f16, tag="enc_t")
    for j in range(NSC):
        op = psT.tile([P, H], f32, tag="pt")
        nc.tensor.matmul(op[:], covr16[0:1, j * P:(j + 1) * P], wcovr16[0:1, :],
                         start=True, stop=True)
        nc.vector.tensor_add(enc16[:, j * H:(j + 1) * H],
                             enc16[:, j * H:(j + 1) * H], op[:])
    nc.scalar.activation(enc_t[:], enc16[:], AF.Tanh)

    # attn_h dec-part (independent of the attention loop): start pa early
    pa = psOut.tile([T, H], f32, tag="pa")
    for k in range(KT):
        nc.tensor.matmul(pa[:], decT16[:, k * T:(k + 1) * T],
                         wo16[:, (KT + k) * H:(KT + k + 1) * H],
                         start=(k == 0), stop=False)
    nc.tensor.matmul(pa[:], ones16[0:1, :], bor16[0:1, :], start=False,
                     stop=False)


    # ---- scoresT accumulator; zero the bank once
    scoresT = psOut.tile([P, NSC * T], f32, tag="scT")
    zrow = pers.tile([1, NSC * T], f16, tag="zrow")
    nc.vector.memset(zrow[:], 0.0)
    nc.tensor.matmul(scoresT[:], ones128[0:1, :], zrow[0:1, :],
                     start=True, stop=False)

    # ---- per-cell features + PE contraction
    # series: tanh(x+y) = P + tau*W - tau^2 P W + tau^3 P^2 W   (W = 1-P^2)
    # with taum = -tau and Wm = P^2-1 = -W the pairs are:
    #   (mv, P), (mv*taum, Wm), (mv*taum^2, P*Wm), (mv*taum^3, P^2*Wm)
    n_cells = C_CELLS
    ctrb = pers.tile([P, n_cells], f32, tag="ctrb")
    for ci in range(n_cells):
        nc.vector.memset(ctrb[:, ci:ci + 1], float(DELTA * (ci - CH)))
    # per-cell series degree: outer cells carry ~no probability mass
    DS = {0: 1, 1: 1, 2: 2, 3: 3, 4: 3, 5: 3, 6: 2, 7: 1, 8: 1}
    # Square on DVE for these cells (engine balance); ACT otherwise
    SQ_DVE = {0, 1, 7, 8, 2}
    # emission order center-out so the last cell has the shortest chain
    order = [4, 3, 5, 2, 6, 1, 7, 0, 8]
    for oi, ci in enumerate(order):
        D = DS[ci] if n_cells == 9 else 3
        Pt = rt.tile([P, KT * S], f16, tag="P", name=f"P{ci}")
        nc.scalar.activation(Pt[:], a2T[:], AF.Tanh, bias=ctrb[:, ci:ci + 1])
        P2 = rt.tile([P, KT * S], f16, tag="P2", name=f"P2{ci}")
        if ci in SQ_DVE and n_cells == 9:
            nc.vector.tensor_tensor(P2[:], Pt[:], Pt[:], ALU.mult)
        else:
            nc.scalar.activation(P2[:], Pt[:], AF.Square)
        rights = [Pt, P2]
        if D >= 2:
            P3 = rt.tile([P, KT * S], f16, tag="P3", name=f"P3{ci}")
            nc.vector.tensor_tensor(P3[:], Pt[:], P2[:], ALU.mult)
            rights.append(P3)
        if D >= 3:
            P4 = rt.tile([P, KT * S], f16, tag="P4", name=f"P4{ci}")
            nc.vector.tensor_tensor(P4[:], P2[:], P2[:], ALU.mult)
            rights.append(P4)

        # left features: mask on DVE (tiny), mask*u_k products on GPSIMD
        mask = lt.tile([P, KT * T], f16, tag="mask", name=f"mask{ci}")
        nc.vector.tensor_scalar(mask[:], cellf[:], float(ci - CH), None,
                                ALU.is_equal)
        ls = []
        for k4 in range(D + 1):
            lk = lt.tile([P, KT * T], f16, tag=f"l{k4}", name=f"l{k4}_{ci}")
            nc.gpsimd.tensor_tensor(lk[:], mask[:], upow[k4][:], ALU.mult)
            ls.append(lk)

        last_cell = (oi == len(order) - 1)
        # signs: scores += l0*P + l1*P2 + l2*P3 - l3*P4; fold the minus by
        # negating uD once? uD pairs with P4 only, so negate lk on GPSIMD:
        # simpler: uD holds +v*taum^3 = -v*tau^3 and the series term is
        # -tau^3*P4 = +taum^3*P4, so (l3, P4) adds with PLUS sign. Check:
        # series: ... + tau^2 P^3 - tau^3 P^4; l2 = v tau^2 (vt2 = v taum^2
        # = v tau^2, ok), l3 = v taum^3 = -v tau^3 -> l3*P4 = -v tau^3 P4 ok.
        for pi, (Lt, Gt) in enumerate(zip(ls, rights)):
            for k in range(KT):
                for c in range(NSC):
                    stop = (last_cell and pi == D and k == KT - 1
                            and c == NSC - 1)
                    nc.tensor.matmul(
                        scoresT[:, c * T:(c + 1) * T],
                        Gt[:, k * S + c * P:k * S + (c + 1) * P],
                        Lt[:, k * T:(k + 1) * T],
                        start=False, stop=stop)

    # ---- epilogue: transpose, softmax, c, attn_h
    scoresT_sb = pers.tile([P, NSC * T], f32, tag="scT_sb")
    nc.vector.tensor_copy(scoresT_sb[:], scoresT[:])
    if "scdbg" in dout:
        nc.sync.dma_start(out=dout["scdbg"][:], in_=scoresT_sb[:])
    scores = psOut.tile([T, S], f32, tag="out512")
    for c in range(NSC):
        nc.tensor.transpose(scores[:, c * P:(c + 1) * P],
                            scoresT_sb[:, c * T:(c + 1) * T], eye128[:])

    # softmax over s; |scores| is small so exp without max-shift is safe
    align_sb = pers.tile([T, S], f32, tag="align_sb")
    sums = pers.tile([T, 1], f32, tag="sums")
    nc.scalar.activation(align_sb[:], scores[:], AF.Exp, accum_out=sums[:])
    recips = pers.tile([T, 1], f32, tag="recips")
    nc.vector.reciprocal(recips[:], sums[:])
    nc.vector.tensor_scalar_mul(align_sb[:], align_sb[:], recips[:])
    nc.sync.dma_start(out=dout["align"][:], in_=align_sb[:])

    # alignT via PE transpose, fp16 copies for the c-matmul
    alignT = []
    for j in range(NSC):
        pt = psSm.tile([P, T], f32, tag="ps")
        nc.tensor.transpose(pt[:], align_sb[:, j * P:(j + 1) * P], eye64[:])
        at = pers.tile([P, T], f16, tag=f"alignT{j}")
        nc.vector.tensor_copy(at[:], pt[:])
        alignT.append(at)

    # cT[h, t] = sum_s enc_t[s,h] * alignT[s,t]  (j outer: overlaps transposes)
    pc = psOut.tile([P, KT * T], f32, tag="pc")
    nc.tensor.matmul(pc[:], ones128[0:1, :], zrow[0:1, :], start=True, stop=False)
    for j in range(NSC):
        for m in range(KT):
            nc.tensor.matmul(pc[:, m * T:(m + 1) * T],
                             enc_t[:, j * H + m * P:j * H + (m + 1) * P],
                             alignT[j][:], start=False,
                             stop=(j == NSC - 1 and m == KT - 1))
    cT = []
    for m in range(KT):
        ct = pers.tile([P, T], f16, tag=f"cT{m}")
        nc.vector.tensor_copy(ct[:], pc[:, m * T:(m + 1) * T])
        cT.append(ct)

    # attn_h = [c, dec] @ Wo + bo  (dec part + bias accumulated early above)
    for k in range(KT):
        nc.tensor.matmul(pa[:], cT[k][:], wo16[:, k * H:(k + 1) * H],
                         start=False, stop=(k == KT - 1))
    attn_sb = pers.tile([T, H], f32, tag="attn_sb")
    nc.vector.tensor_copy(attn_sb[:], pa[:])
    nc.sync.dma_start(out=dout["attn_h"][:], in_=attn_sb[:])


def build(debug_scores=False):
    key = ("cells", debug_scores, C_CELLS, DELTA)
    if key in _BUILT:
        return _BUILT[key]
    from contextlib import ExitStack

    import concourse.bacc as bacc
    import concourse.mybir as mybir
    import concourse.tile as tile

    f32 = mybir.dt.float32
    f16 = mybir.dt.float16
    nc = bacc.Bacc("TRN2", target_bir_lowering=False, debug=False)
    in_specs = [
        ("cov16", [1, S], f16), ("wcov16", [1, H], f16),
        ("encT16", [P, KT * S], f16), ("wc16", [P, KT * H], f16),
        ("decT16", [P, KT * T], f16), ("wq16", [P, KT * H], f16),
        ("bq16", [1, H], f16), ("vrep16", [P, KT * T], f16),
        ("enc16", [P, KT * H], f16), ("wo16", [P, 2 * KT * H], f16),
        ("bo16", [1, H], f16), ("eye64", [T, T], f32), ("eye128", [P, P], f32),
    ]
    out_specs = [("attn_h", [T, H], f32), ("align", [T, S], f32)]
    if debug_scores:
        out_specs.append(("scdbg", [P, NSC * T], f32))
    din = {n: nc.declare_dram_parameter(n, s, d, isOutput=False)
           for n, s, d in in_specs}
    dout = {n: nc.declare_dram_parameter(n, s, d, isOutput=True)
            for n, s, d in out_specs}
    with ExitStack() as ctx:
        tc = ctx.enter_context(tile.TileContext(nc))
        _emit(nc, tc, ctx, din, dout)
    nc.compile()
    _BUILT[key] = nc
    return nc


def _merge(x, chunks):
    """[chunks*P, F] -> [P, chunks*F] fp16 (partition-major merge)."""
    cp, F = x.shape
    assert cp == chunks * P
    return np.ascontiguousarray(
        x.reshape(chunks, P, F).transpose(1, 0, 2).reshape(P, chunks * F)
    ).astype(np.float16)


def prep_core_inputs(inputs):
    """Host-side shard: per-core input dicts (core b <- batch element b)."""
    dec = np.asarray(inputs["attn_dec_state"], np.float32)   # [T,B,H]
    encr = np.asarray(inputs["attn_enc_state"], np.float32)  # [S,B,H]
    cov = np.asarray(inputs["attn_coverage"], np.float32)    # [B,S]
    Wq = np.asarray(inputs["Wq"], np.float32)
    Wc = np.asarray(inputs["Wc"], np.float32)
    Wo = np.asarray(inputs["Wo"], np.float32)
    v = np.asarray(inputs["v"], np.float32)
    bq = np.asarray(inputs["bq"], np.float32)[None, :]
    bo = np.asarray(inputs["bo"], np.float32)[None, :]
    wcov = np.asarray(inputs["wcov"], np.float32)[None, :]
    vrep = np.zeros((P, KT * T), np.float32)
    for k in range(KT):
        vrep[:, k * T:(k + 1) * T] = v[k * P:(k + 1) * P][:, None]
    shared = dict(
        wq16=_merge(Wq, KT), wc16=_merge(Wc, KT), wo16=_merge(Wo, 2 * KT),
        vrep16=vrep.astype(np.float16), wcov16=wcov.astype(np.float16),
        bq16=bq.astype(np.float16), bo16=bo.astype(np.float16),
        eye64=np.eye(T, dtype=np.float32), eye128=np.eye(P, dtype=np.float32),
    )
    maps = []
    for b in range(B):
        e = np.ascontiguousarray(encr[:, b, :])           # [S,H]
        maps.append(dict(
            decT16=_merge(np.ascontiguousarray(dec[:, b, :].T), KT),
            enc16=_merge(e, NSC),
            encT16=_merge(np.ascontiguousarray(e.T), KT),
            cov16=np.ascontiguousarray(cov[b][None, :]).astype(np.float16),
            **shared,
        ))
    return maps


def kernel(**inputs):
    global LAST_RESULT
    nc = build()
    in_maps = prep_core_inputs(inputs)
    from concourse.bass_utils import run_bass_kernel_spmd

    trace = os.environ.get("ATTN_TRACE", "0") == "1"
    res = run_bass_kernel_spmd(nc, in_maps, list(range(B)), trace=trace)
    LAST_RESULT = res
    attn_h = np.stack([res.results[i]["attn_h"] for i in range(B)], axis=1)
    align = np.stack([res.results[i]["align"] for i in range(B)], axis=1)
    return attn_h, align
